# revision 1
# baseline (speedup 1.0000x reference)
"""Causal linear attention layer (elu+1 feature map) on 8 Trainium2 NeuronCores.

Sharding: batch x head-group parallel. 32 (batch, head) jobs -> 8 cores:
core c handles batch b = c // 4 and heads 4*(c%4) .. 4*(c%4)+3, i.e. a
256-channel slice of the projections. Each core:
  - computes its slice of the q/k/v projections (contraction over the full
    d_model, weights pre-sliced + pre-transposed on host, bf16),
  - applies the elu+1 feature map to q, k,
  - runs chunked causal linear attention (chunk = 128 positions) with a
    per-head [64, 64+1] fp32 state accumulated in PSUM (the +1 column
    carries the running sum of K for the normalizer),
  - projects through the matching 256-column slice of Wo, writing a
    transposed partial output [1024, seq] fp32.
Host sums the 4 partials per batch, transposes, and adds bo.
"""

import numpy as np
import ml_dtypes

import concourse.bass as bass
import concourse.mybir as mybir
import concourse.tile as tile
from bass_rust import SyncInfo

BF16 = mybir.dt.bfloat16
F32 = mybir.dt.float32
AF = mybir.ActivationFunctionType
OP = mybir.AluOpType

D_MODEL = 1024
N_HEAD = 16
HD = 64
B = 2
L = 2048
EPS = 1e-6
N_CORES = 8
HPC = 4                 # heads per core
CH = HPC * HD           # 256 channel slice per core
KT_N = D_MODEL // 128   # 8 k-tiles for q/k projections
KTA_N = KT_N + 1        # 9 k-tiles for the bias-augmented v projection
CHUNK = 128


def build_nc(seq=L, stop_after="all"):
    nc = _build_nc_inner(seq, stop_after)
    _split_multi_waits(nc)
    return nc


def _build_nc_inner(seq=L, stop_after="all"):
    """Build the per-core Bass program (SPMD: all 8 cores run this)."""
    assert seq % 512 == 0
    seqt = seq // CHUNK          # chunks / seq tiles
    nseq = seq // 512            # 512-wide column chunks for projections

    nc = bass.Bass("TRN2", target_bir_lowering=False)

    xqT = nc.dram_tensor("xqT", [D_MODEL, seq], BF16, kind="ExternalInput")
    xkT = nc.dram_tensor("xkT", [D_MODEL, seq], BF16, kind="ExternalInput")
    xvT = nc.dram_tensor("xvT", [D_MODEL + 128, seq], BF16, kind="ExternalInput")
    wqT = nc.dram_tensor("wqT", [D_MODEL, CH], BF16, kind="ExternalInput")
    wkT = nc.dram_tensor("wkT", [D_MODEL, CH], BF16, kind="ExternalInput")
    wvT = nc.dram_tensor("wvT", [D_MODEL + 128, CH], BF16, kind="ExternalInput")
    woT = nc.dram_tensor("woT", [CH, D_MODEL], BF16, kind="ExternalInput")
    bqe_d = nc.dram_tensor("bqe", [CH, 1], F32, kind="ExternalInput")   # scaled bq
    bqp_d = nc.dram_tensor("bqp", [CH, 1], F32, kind="ExternalInput")   # scaled bq + 1
    bke_d = nc.dram_tensor("bke", [CH, 1], F32, kind="ExternalInput")
    bkp_d = nc.dram_tensor("bkp", [CH, 1], F32, kind="ExternalInput")
    mask_d = nc.dram_tensor("mask4", [128, 512], BF16, kind="ExternalInput")  # causal mask, 4x tiled
    ident_d = nc.dram_tensor("ident", [128, 128], BF16, kind="ExternalInput")
    identa_d = nc.dram_tensor("identA", [128, 64], BF16, kind="ExternalInput")
    outT = nc.dram_tensor("outT", [D_MODEL, seq], F32, kind="ExternalOutput")

    with tile.TileContext(nc) as tc:
        with (
            tc.tile_pool(name="res", bufs=1) as rp,
            tc.tile_pool(name="work", bufs=3) as wp,
            tc.tile_pool(name="ps", bufs=2, space="PSUM") as pp,
        ):
            # ---------------- constant / weight loads ----------------
            mask = rp.tile([128, 512], BF16, tag="mask")
            nc.sync.dma_start(mask[:], mask_d[:])
            ident = rp.tile([128, 128], BF16, tag="ident")
            nc.sync.dma_start(ident[:], ident_d[:])
            identa = rp.tile([128, 64], BF16, tag="identa")
            nc.sync.dma_start(identa[:], identa_d[:])

            wq = [rp.tile([128, CH], BF16, tag=f"wq{k}", name=f"wq{k}") for k in range(KT_N)]
            wk = [rp.tile([128, CH], BF16, tag=f"wk{k}", name=f"wk{k}") for k in range(KT_N)]
            wv = [rp.tile([128, CH], BF16, tag=f"wv{k}", name=f"wv{k}") for k in range(KTA_N)]
            wo = [rp.tile([128, D_MODEL], BF16, tag=f"wo{t}", name=f"wo{t}") for t in range(2)]
            for k in range(KT_N):
                nc.sync.dma_start(wq[k][:], wqT[k * 128 : (k + 1) * 128, :])
                nc.sync.dma_start(wk[k][:], wkT[k * 128 : (k + 1) * 128, :])
            for k in range(KTA_N):
                nc.sync.dma_start(wv[k][:], wvT[k * 128 : (k + 1) * 128, :])
            for t in range(2):
                nc.sync.dma_start(wo[t][:], woT[t * 128 : (t + 1) * 128, :])

            bqe, bqp, bke, bkp = (
                [rp.tile([128, 1], F32, tag=f"b{i}{t}", name=f"b{i}{t}") for t in range(2)]
                for i in range(4)
            )
            for t in range(2):
                sl = slice(t * 128, (t + 1) * 128)
                nc.sync.dma_start(bqe[t][:], bqe_d[sl, :])
                nc.sync.dma_start(bqp[t][:], bqp_d[sl, :])
                nc.sync.dma_start(bke[t][:], bke_d[sl, :])
                nc.sync.dma_start(bkp[t][:], bkp_d[sl, :])

            # ---------------- activation loads ----------------
            xq = [rp.tile([128, seq], BF16, tag=f"xq{k}", name=f"xq{k}") for k in range(KT_N)]
            xk = [rp.tile([128, seq], BF16, tag=f"xk{k}", name=f"xk{k}") for k in range(KT_N)]
            xv = [rp.tile([128, seq], BF16, tag=f"xv{k}", name=f"xv{k}") for k in range(KTA_N)]
            for k in range(KT_N):
                nc.sync.dma_start(xq[k][:], xqT[k * 128 : (k + 1) * 128, :])
                nc.sync.dma_start(xk[k][:], xkT[k * 128 : (k + 1) * 128, :])
            for k in range(KTA_N):
                nc.sync.dma_start(xv[k][:], xvT[k * 128 : (k + 1) * 128, :])

            # ---------------- q/k projections + elu+1 features ----------------
            # Feature map: elu(y)+1 = max(y+1, min(exp(y), 1)).
            QT = [rp.tile([128, seq], BF16, tag=f"QT{t}", name=f"QT{t}") for t in range(2)]
            KTf = [rp.tile([128, seq], BF16, tag=f"KTf{t}", name=f"KTf{t}") for t in range(2)]
            for X, W, be, bp, OUT in (
                (xq, wq, bqe, bqp, QT),
                (xk, wk, bke, bkp, KTf),
            ):
                for mt in range(2):
                    for n in range(nseq):
                        ncols = slice(n * 512, (n + 1) * 512)
                        ps = pp.tile([128, 512], F32, tag="ps")
                        for k in range(KT_N):
                            nc.tensor.matmul(
                                ps[:],
                                W[k][:, mt * 128 : (mt + 1) * 128],
                                X[k][:, ncols],
                                start=(k == 0),
                                stop=(k == KT_N - 1),
                            )
                        ex = wp.tile([128, 512], BF16, tag="ex")
                        nc.scalar.activation(ex[:], ps[:], AF.Exp, bias=be[mt][:, 0:1])
                        exc = wp.tile([128, 512], BF16, tag="exc")
                        nc.vector.tensor_scalar_min(exc[:], ex[:], 1.0)
                        nc.vector.scalar_tensor_tensor(
                            OUT[mt][:, ncols], ps[:], bp[mt][:, 0:1], exc[:],
                            op0=OP.add, op1=OP.max,
                        )

            if stop_after == "proj":
                return _finish(nc)
            # ---------------- v projection (bias via augmented row) ----------------
            # vst[m]: [128, 260] = 4 blocks of (64 v-cols + 1 ones-col) per head.
            vst = [rp.tile([128, HPC * (HD + 1)], BF16, tag=f"vst{m}", name=f"vst{m}") for m in range(seqt)]
            for m in range(seqt):
                nc.gpsimd.memset(vst[m][:], 1.0)
                ps = pp.tile([128, CH], F32, tag="ps")
                for k in range(KTA_N):
                    nc.tensor.matmul(
                        ps[:],
                        xv[k][:, m * 128 : (m + 1) * 128],
                        wv[k][:],
                        start=(k == 0),
                        stop=(k == KTA_N - 1),
                    )
                nc.vector.tensor_copy(
                    vst[m].rearrange("p (h e) -> p h e", e=HD + 1)[:, :, 0:HD],
                    ps.rearrange("p (h e) -> p h e", e=HD)[:, :, :],
                )

            if stop_after == "vproj":
                return _finish(nc)
            # ---------------- K natural layout via PE transpose ----------------
            knat = [rp.tile([128, CH], BF16, tag=f"knat{m}", name=f"knat{m}") for m in range(seqt)]
            for m in range(seqt):
                for t in range(2):
                    kp = pp.tile([128, 128], BF16, tag="kp", bufs=1)
                    nc.tensor.matmul(
                        kp[:],
                        KTf[t][:, m * 128 : (m + 1) * 128],
                        ident[:],
                        is_transpose=True,
                        start=True,
                        stop=True,
                    )
                    nc.scalar.activation(knat[m][:, t * 128 : (t + 1) * 128], kp[:], AF.Copy)

            if stop_after == "ktrans":
                return _finish(nc)
            # ---------------- chunked causal linear attention ----------------
            s_acc = [rp.tile([64, HPC * (HD + 1)], F32, tag=f"sacc{i}", name=f"sacc{i}") for i in range(2)]
            onat = [rp.tile([128, CH], BF16, tag=f"onat{m}", name=f"onat{m}") for m in range(seqt)]
            OTW = [
                [rp.tile([128, 512], BF16, tag=f"OTW{n}_{t}", name=f"OTW{n}_{t}") for t in range(2)]
                for n in range(seqt // 4)
            ]
            for m in range(seqt):
                cc = slice(m * 128, (m + 1) * 128)
                prev, cur = s_acc[(m + 1) % 2], s_acc[m % 2]

                # A^T = K_c Q_c^T in [s, t] layout, one psum bank per head
                atm = wp.tile([128, 512], BF16, tag="atm")
                for h in range(HPC):
                    t, off = h // 2, (h % 2) * 64
                    at = pp.tile([128, 128], F32, tag="at", bufs=2)
                    nc.tensor.matmul(
                        at[:],
                        KTf[t][off : off + 64, cc],
                        QT[t][off : off + 64, cc],
                        start=True,
                        stop=True,
                    )
                    nc.vector.tensor_tensor(
                        atm[:, h * 128 : (h + 1) * 128], at[:],
                        mask[:, 0:128], op=OP.mult,
                    )

                if m > 0:
                    # bf16 snapshot of S duplicated into both partition halves
                    ssb = wp.tile([128, 2 * (HD + 1)], BF16, tag="ssb")
                    spv = prev.rearrange("p (h e) -> p h e", e=HD + 1)
                    sbv0 = ssb[0:64, :].rearrange("p (h e) -> p h e", e=HD + 1)
                    sbv1 = ssb[64:128, :].rearrange("p (h e) -> p h e", e=HD + 1)
                    nc.vector.tensor_copy(sbv0[:, :, :], spv[:, 0::2, :])
                    nc.vector.tensor_copy(sbv1[:, :, :], spv[:, 1::2, :])

                # per head: numerator+normalizer accumulate in one [128, 65] bank
                for h in range(HPC):
                    t, off = h // 2, (h % 2) * 64
                    on = pp.tile([128, HD + 1], F32, tag="on", name=f"on{m}_{h}", bufs=2)
                    nc.tensor.matmul(
                        on[:],
                        atm[:, h * 128 : (h + 1) * 128],
                        vst[m][:, h * (HD + 1) : (h + 1) * (HD + 1)],
                        start=True,
                        stop=(m == 0),
                    )
                    if m > 0:
                        cs = slice((h // 2) * (HD + 1), (h // 2 + 1) * (HD + 1))
                        nc.tensor.matmul(
                            on[:],
                            QT[t][off : off + 64, cc],
                            ssb[off : off + 64, cs],
                            start=False,
                            stop=True,
                        )
                    zr = wp.tile([128, 1], F32, tag="zr")
                    nc.vector.tensor_scalar_add(zr[:], on[:, HD : HD + 1], EPS)
                    nc.vector.reciprocal(zr[:], zr[:])
                    nc.vector.tensor_scalar_mul(
                        onat[m][:, h * HD : (h + 1) * HD],
                        on[:, 0:HD],
                        zr[:, 0:1],
                    )

                # state += K_c^T V_c via single-shot psum + SBUF ping-pong add
                for h in range(HPC):
                    es = slice(h * (HD + 1), (h + 1) * (HD + 1))
                    st = pp.tile([64, HD + 1], F32, tag="st", name=f"st{m}_{h}", bufs=1)
                    nc.tensor.matmul(
                        st[:],
                        knat[m][:, h * HD : (h + 1) * HD],
                        vst[m][:, es],
                        start=True,
                        stop=True,
                    )
                    if m == 0:
                        nc.vector.tensor_copy(cur[:, es], st[:])
                    else:
                        nc.vector.tensor_tensor(cur[:, es], prev[:, es], st[:], op=OP.add)

                # O -> [ch, seq] transpose for this chunk
                for t in range(2):
                    op_ = pp.tile([128, 128], BF16, tag="kp", bufs=1)
                    nc.tensor.matmul(
                        op_[:],
                        onat[m][:, t * 128 : (t + 1) * 128],
                        ident[:],
                        is_transpose=True,
                        start=True,
                        stop=True,
                    )
                    nc.vector.tensor_copy(
                        OTW[m // 4][t][:, (m % 4) * 128 : (m % 4 + 1) * 128],
                        op_[:],
                    )

                # once a 512-wide window of OT is complete, project it
                if m % 4 == 3:
                    n = m // 4
                    for j in range(D_MODEL // 128):
                        po = pp.tile([128, 512], F32, tag="ps")
                        for t in range(2):
                            nc.tensor.matmul(
                                po[:],
                                wo[t][:, j * 128 : (j + 1) * 128],
                                OTW[n][t][:],
                                start=(t == 0),
                                stop=(t == 1),
                            )
                        oev = wp.tile([128, 512], F32, tag="oev")
                        nc.scalar.activation(oev[:], po[:], AF.Copy)
                        nc.sync.dma_start(
                            outT[j * 128 : (j + 1) * 128, n * 512 : (n + 1) * 512],
                            oev[:],
                        )

            if stop_after == "attn":
                return _finish(nc)

    return nc
    return nc


def _finish(nc):
    return nc


def _split_multi_waits(nc, max_waits=1):
    """This toolchain's walrus encodes at most one sync-wait per instruction;
    hoist extra waits onto single-wait NoOps on the same engine queue."""
    for f in nc.m.functions:
        for blk in f.blocks:
            insts = list(blk.instructions)
            out, changed = [], False
            for inst in insts:
                si = inst.sync_info
                if si is not None and si.on_wait and len(si.on_wait) > max_waits:
                    waits = list(si.on_wait)
                    hoist, keep = waits[:-max_waits], waits[-max_waits:]
                    for j, w in enumerate(hoist):
                        nop = mybir.InstNoOp(name=f"{inst.name}-ws{j}")
                        nop.engine = inst.engine
                        nop.sync_info = SyncInfo(on_wait=[w], on_update=[])
                        nc.register_instruction(nop)
                        out.append(nop)
                    inst.sync_info = SyncInfo(on_wait=keep, on_update=list(si.on_update))
                    changed = True
                out.append(inst)
            if changed:
                blk.instructions = out


def host_prepare(querys, keys, values, Wq, bq, Wk, bk, Wv, bv, Wo, bo, seq=L):
    """Build the 8 per-core input maps from the full-size fp32 inputs."""
    bf = ml_dtypes.bfloat16
    scale = HD ** -0.5
    mask = np.triu(np.ones((128, 128), np.float32))        # [s, t], keep s <= t
    mask4 = np.tile(mask, (1, 4)).astype(bf)
    ident = np.eye(128, dtype=bf)
    identa = np.vstack([np.eye(64), np.eye(64)]).astype(bf)

    xT = {}
    for b in range(B):
        xT[("q", b)] = np.ascontiguousarray(querys[b, :seq].T).astype(bf)
        xT[("k", b)] = np.ascontiguousarray(keys[b, :seq].T).astype(bf)
        xv = np.zeros((D_MODEL + 128, seq), np.float32)
        xv[:D_MODEL] = values[b, :seq].T
        xv[D_MODEL] = 1.0
        xT[("v", b)] = xv.astype(bf)

    in_maps = []
    for c in range(N_CORES):
        b, g = c // 4, c % 4
        ch = slice(g * CH, (g + 1) * CH)
        wvT = np.zeros((D_MODEL + 128, CH), np.float32)
        wvT[:D_MODEL] = Wv[ch].T
        wvT[D_MODEL] = bv[ch]
        bqs = (bq[ch] * scale).astype(np.float32)
        in_maps.append({
            "xqT": xT[("q", b)],
            "xkT": xT[("k", b)],
            "xvT": xT[("v", b)],
            "wqT": np.ascontiguousarray((Wq[ch] * scale).T).astype(bf),
            "wkT": np.ascontiguousarray(Wk[ch].T).astype(bf),
            "wvT": wvT.astype(bf),
            "woT": np.ascontiguousarray(Wo[:, ch].T).astype(bf),
            "bqe": bqs[:, None],
            "bqp": (bqs + 1.0)[:, None],
            "bke": bk[ch].astype(np.float32)[:, None],
            "bkp": (bk[ch] + 1.0).astype(np.float32)[:, None],
            "mask4": mask4,
            "ident": ident,
            "identA": identa,
        })
    return in_maps


def gather_output(results, bo, seq=L):
    """Sum per-core transposed partials, transpose back, add bo."""
    out = np.empty((B, seq, D_MODEL), np.float32)
    for b in range(B):
        acc = results[4 * b]["outT"].copy()
        for g in range(1, 4):
            acc += results[4 * b + g]["outT"]
        out[b] = acc.T + bo[None, :]
    return out


_nc_cache = {}


def kernel(**inputs):
    from concourse.bass_utils import run_bass_kernel_spmd

    if L not in _nc_cache:
        _nc_cache[L] = build_nc(L)
    nc = _nc_cache[L]
    in_maps = host_prepare(**inputs)
    res = run_bass_kernel_spmd(nc, in_maps, list(range(N_CORES)))
    return gather_output([res.results[c] for c in range(N_CORES)],
                         np.asarray(inputs["bo"], np.float32))

